# revision 9
# baseline (speedup 1.0000x reference)
"""Causal self-attention TRN2 kernel, tensor-parallel over heads on 8 NeuronCores.

Model (N=4096 tokens, D=2048, H=16 heads, HD=128):
    q = x @ Wq.T + bq ; k = x @ Wk.T + bk ; v = x @ Wv.T + bv   (per head)
    attn = softmax(q k^T / sqrt(HD) + causal_mask)
    y = concat_h(attn @ v) @ Wo.T + bo
Sharding: core c owns heads {2c, 2c+1}; host sums the 8 partial outputs.

Per-core layout (same as the v1 kernel): x fed transposed, qT/kT produced
(HD x N) fp16, scores computed transposed sT[k,q] so PV needs no transposes,
v stored (n, hd) via PE transposes, v-bias folded into the output bias.

v2 performance changes:
  * Phase 1 runs the 16-chunk contraction per OUTPUT (q0,q1,k0,k1,v0,v1)
    instead of per chunk, so each output's PSUM bank frees ~17us before its
    next use -- removes the per-stripe psum-reuse bubble.  k biases are
    applied on DVE (per-partition scalar) in parallel with q's ACT path.
  * The softmax rowsum no longer streams every exp tile through the PE
    (was ~14%% of PE work): exp tiles are accumulated over key blocks on
    DVE (head 0) / gpsimd (head 1) into a fp16 pacc tile, and a single
    512-col ones-matmul per (qb, head) contracts the 128 lanes.
  * Scores are computed into [128,1024] 2-bank PSUM tiles (two key blocks
    per tile) and exp'd with ONE ACT op per pair -- amortizes ACT fixed
    overhead; ACT would otherwise pace the attention once the rowsum
    matmuls are gone.  Diagonal blocks share paired tiles too, with
    precomputed combined triangular masks (one DVE add per 2 blocks).
  * 1/rowsum via reciprocal_approx_fast (~5x faster than reciprocal,
    ~18 correct bits -- plenty for a softmax denominator).
  * y is written fp16 (host accumulates in fp32), one DMA per 128-row slab.
  * Output-projection bias adds alternate DVE/gpsimd; outproj is emitted
    one 128-row group per pipeline slot to avoid 7us PE bursts starving ACT.
"""

from contextlib import ExitStack

import numpy as np

import concourse.bass as bass
import concourse.tile as tile
from concourse import bacc
from concourse import mybir
from concourse.bass_utils import run_bass_kernel_spmd
from concourse.masks import make_identity

N, D, H, HD = 4096, 2048, 16, 128
NCORES = 8
HPC = H // NCORES            # heads per core (2)
CD = HPC * HD                # per-core head-dim slice (256)
SCALE = 1.0 / float(np.sqrt(HD))
NEG = -1e9

QB = 512                     # query block (free dim of moving operands)
KB = 128                     # key block (partition dim of scores)
NQB = N // QB                # 8
KC = D // 128                # contraction chunks for projections (16)

F32 = mybir.dt.float32
F16 = mybir.dt.float16


def build_nc(causal: bool = True) -> bass.Bass:
    nc = bacc.Bacc(None)

    xT = nc.declare_dram_parameter("xT", [D, N], F16, isOutput=False)
    wqT = nc.declare_dram_parameter("wqT", [D, CD], F16, isOutput=False)
    wkT = nc.declare_dram_parameter("wkT", [D, CD], F16, isOutput=False)
    wvT = nc.declare_dram_parameter("wvT", [D, CD], F16, isOutput=False)
    woT = nc.declare_dram_parameter("woT", [CD, D], F16, isOutput=False)
    bq = nc.declare_dram_parameter("bq", [CD], F32, isOutput=False)
    bk = nc.declare_dram_parameter("bk", [CD], F32, isOutput=False)
    maskT = None
    if not causal:
        maskT = nc.declare_dram_parameter("maskT", [N, N], F32, isOutput=False)
    y = nc.declare_dram_parameter("y", [N, D], F16, isOutput=True)

    with tile.TileContext(nc) as tc, ExitStack() as ctx:
        persist = ctx.enter_context(tc.tile_pool(name="persist", bufs=1))

        # -------- setup: phase-1-critical DMAs first ----------------------
        bq_sb = persist.tile([128, HPC], F32, tag="bq")
        bk_sb = persist.tile([128, HPC], F32, tag="bk")
        nc.sync.dma_start(out=bq_sb[:], in_=bq[:].rearrange("(h p) -> p h", p=128))
        nc.sync.dma_start(out=bk_sb[:], in_=bk[:].rearrange("(h p) -> p h", p=128))

        wproj = ctx.enter_context(tc.tile_pool(name="wproj", bufs=1))
        wq_sb = wproj.tile([128, KC * CD], F16, tag="wq")
        wk_sb = wproj.tile([128, KC * CD], F16, tag="wk")
        wv_sb = wproj.tile([128, KC * CD], F16, tag="wv")
        for w_sb, w_dram in ((wq_sb, wqT), (wk_sb, wkT), (wv_sb, wvT)):
            for c in range(KC):
                nc.sync.dma_start(
                    out=w_sb[:, c * CD:(c + 1) * CD],
                    in_=w_dram[c * 128:(c + 1) * 128, :],
                )

        # phase-2 weights; transfers overlap phase-1 compute.  The output
        # bias is added on the HOST after summing the 8 partials, so the
        # outproj evacuation is a plain psum->fp16 copy.
        wo_sb = persist.tile([128, HPC * D], F16, tag="wo")
        nc.sync.dma_start(
            out=wo_sb[:].rearrange("p (h d) -> p h d", h=HPC),
            in_=woT[:].rearrange("(h p) d -> p h d", p=128),
        )

        ident = persist.tile([128, 128], F16, tag="ident")
        # ones stationary: rowsum matmul broadcasts the lane-sum to all
        # 128 output partitions at no extra cost
        ones = persist.tile([128, 128], F16, tag="ones")
        nc.vector.memset(ones[:], 1.0)
        # combined triangular masks for the paired diagonal blocks:
        # m[k, seg*512 + t] = 0 if t >= k else NEG for each member segment
        m1 = m2 = None
        if causal:
            m1 = persist.tile([128, QB + 384], F32, tag="m1")   # widths 512,384
            m2 = persist.tile([128, QB + 128], F32, tag="m2")   # widths 256,128
            nc.vector.memset(m1[:], 0.0)
            nc.vector.memset(m2[:], 0.0)

        def emit_setup_selects():
            # gpsimd setup ops, emitted after the first stripe's x DMAs so
            # they don't block the gpsimd queue head
            make_identity(nc, ident[:])
            if causal:
                for mt, spans in ((m1, (QB, 384)), (m2, (256, 128))):
                    for seg, w in enumerate(spans):
                        nc.gpsimd.affine_select(
                            out=mt[:, seg * QB:seg * QB + w],
                            in_=mt[:, seg * QB:seg * QB + w],
                            compare_op=mybir.AluOpType.is_ge,
                            fill=NEG,
                            base=0,
                            pattern=[[1, w]],
                            channel_multiplier=-1,
                        )

        # PE warm-up: dependency-free matmuls fill the DMA-startup window
        # and push the HAM clock gate to full rate before real work
        warm_sb = persist.tile([128, QB], F16, tag="warm")
        nc.vector.memset(warm_sb[:], 0.0)

        # Persistent activations: qT/kT per head (HD x N) fp16; v per head
        # stored (128, 32*128) with free = (n_block, hd) i.e. (N x HD) layout.
        qT = [persist.tile([128, N], F16, tag=f"qT{h}", name=f"qT{h}")
              for h in range(HPC)]
        kT = [persist.tile([128, N], F16, tag=f"kT{h}", name=f"kT{h}")
              for h in range(HPC)]
        v_sb = [persist.tile([128, N], F16, tag=f"v{h}", name=f"v{h}")
                for h in range(HPC)]

        # ---------------- phase 1: QKV projections ----------------------
        xin = ctx.enter_context(tc.tile_pool(name="xin", bufs=24))
        vtpool = ctx.enter_context(tc.tile_pool(name="vt", bufs=3))
        with ExitStack() as p1ps:
            qkv_ps = p1ps.enter_context(
                tc.tile_pool(name="qkv_ps", bufs=6, space="PSUM"))
            tp_ps = p1ps.enter_context(
                tc.tile_pool(name="tp_ps", bufs=2, space="PSUM"))

            warm_ps = qkv_ps.tile([128, QB], F32, tag="qkv", name="warm_ps")
            for wi in range(20):
                nc.tensor.matmul(
                    warm_ps[:], lhsT=ones[:], rhs=warm_sb[:],
                    start=(wi == 0), stop=(wi == 19),
                )

            # v-transposes are deferred one output-group so the PE never
            # waits on the DVE psum->sbuf copy feeding them
            deferred = []

            def flush_deferred():
                while deferred:
                    deferred.pop(0)()

            W = {"q": wq_sb, "k": wk_sb, "v": wv_sb}
            for nb in range(NQB):
                xts = []
                for c in range(KC):
                    xt = xin.tile([128, QB], F16, tag="xt", name=f"xt_{nb}_{c}")
                    nc.gpsimd.dma_start(
                        out=xt[:],
                        in_=xT[c * 128:(c + 1) * 128, nb * QB:(nb + 1) * QB],
                    )
                    xts.append(xt)
                if nb == 0:
                    emit_setup_selects()
                outs = [("q", 0), ("q", 1), ("k", 0), ("k", 1),
                        ("v", 0), ("v", 1)]
                if nb == NQB - 1:
                    # last stripe: v first so its deferred transposes flush
                    # under the q/k matmuls instead of stalling at phase end
                    outs = outs[4:] + outs[:4]
                for nm, h in outs:
                    ps = qkv_ps.tile([128, QB], F32, tag="qkv",
                                     name=f"ps_{nm}{h}_{nb}")
                    w_sb = W[nm]
                    for c in range(KC):
                        nc.tensor.matmul(
                            ps[:],
                            lhsT=w_sb[:, c * CD + h * HD: c * CD + (h + 1) * HD],
                            rhs=xts[c][:],
                            start=(c == 0),
                            stop=(c == KC - 1),
                        )
                    flush_deferred()
                    if nm == "q":
                        nc.scalar.activation(
                            out=qT[h][:, nb * QB:(nb + 1) * QB], in_=ps[:],
                            func=mybir.ActivationFunctionType.Identity,
                            bias=bq_sb[:, h:h + 1], scale=1.0,
                        )
                    elif nm == "k":
                        nc.vector.tensor_scalar_add(
                            out=kT[h][:, nb * QB:(nb + 1) * QB], in0=ps[:],
                            scalar1=bk_sb[:, h:h + 1],
                        )
                    else:
                        vt = vtpool.tile([128, QB], F16, tag="vt",
                                         name=f"vt_{nb}_{h}")
                        nc.vector.tensor_copy(out=vt[:], in_=ps[:])

                        def mk(vt=vt, nb=nb, h=h):
                            def go():
                                for s in range(QB // 128):
                                    tp = tp_ps.tile([128, 128], F16, tag="tp",
                                                    name=f"tp_{nb}_{h}_{s}")
                                    nc.tensor.transpose(
                                        tp[:], vt[:, s * 128:(s + 1) * 128],
                                        ident[:],
                                    )
                                    nblk = nb * (QB // 128) + s
                                    nc.vector.tensor_copy(
                                        out=v_sb[h][:, nblk * 128:(nblk + 1) * 128],
                                        in_=tp[:],
                                    )
                            return go
                        deferred.append(mk())
                if nb == NQB - 1:
                    flush_deferred()

        # ---------------- phase 2: attention + output projection --------
        with ExitStack() as p2:
            ptpool = p2.enter_context(tc.tile_pool(name="pt", bufs=4))
            paccp = p2.enter_context(tc.tile_pool(name="pacc", bufs=4))
            otpool = p2.enter_context(tc.tile_pool(name="ot", bufs=4))
            ypool = p2.enter_context(tc.tile_pool(name="yout", bufs=4))
            small = p2.enter_context(tc.tile_pool(name="small", bufs=4))
            mtpool = p2.enter_context(tc.tile_pool(name="mt", bufs=6))
            s_ps = p2.enter_context(
                tc.tile_pool(name="s_ps", bufs=2, space="PSUM"))   # 2x2 banks
            o_ps = p2.enter_context(
                tc.tile_pool(name="o_ps", bufs=2, space="PSUM"))   # 2 banks
            y_ps = p2.enter_context(
                tc.tile_pool(name="y_ps", bufs=2, space="PSUM"))   # 2 banks

            # chunk = up to two key blocks sharing one [128,1024] score tile
            # and ONE exp.  members: (nkb, qoff, sd_off); qoff>0 only on
            # diagonal-straddling blocks (columns below qoff fully masked).
            chunks = []
            for qb in range(NQB):
                nfull = 4 * qb if causal else N // KB
                for h in range(HPC):
                    mem_chunks = []
                    for j in range(0, nfull - 1, 2):
                        mem_chunks.append([(j, 0, 0), (j + 1, 0, QB)])
                    if causal:
                        dg = 4 * qb
                        mem_chunks.append([(dg, 0, 0), (dg + 1, KB, QB)])
                        mem_chunks.append([(dg + 2, 2 * KB, 0),
                                           (dg + 3, 3 * KB, QB)])
                    for ci, mem in enumerate(mem_chunks):
                        chunks.append((qb, h, mem, ci == 0,
                                       ci == len(mem_chunks) - 1))

            pts = {}
            o_psum = {}
            pacc = {}
            oT_sb = {}
            ready = []
            ready_at = {}
            fifo = []

            def emit_front(i, ch):
                qb, h, mem, first, last = ch
                sd = s_ps.tile([128, 2 * QB], F32, tag="s", name=f"s_{i}")
                for nkb, qoff, soff in mem:
                    w = QB - qoff
                    nc.tensor.matmul(
                        sd[:, soff:soff + w],
                        lhsT=kT[h][:, nkb * KB:(nkb + 1) * KB],
                        rhs=qT[h][:, qb * QB + qoff:(qb + 1) * QB],
                        start=True,
                        stop=True,
                    )
                exp_end = mem[-1][2] + (QB - mem[-1][1])
                if causal and mem[0][0] >= 4 * qb:
                    # paired diagonal blocks: one combined-mask add
                    # (PSUM access -> must be DVE, gpsimd cannot touch PSUM)
                    mt = m1 if mem[0][1] == 0 else m2
                    nc.vector.tensor_add(
                        sd[:, :exp_end], sd[:, :exp_end], mt[:, :exp_end])
                elif not causal:
                    for nkb, qoff, soff in mem:
                        mt = mtpool.tile([128, QB], F32, tag="mt",
                                         name=f"mt_{i}_{soff}")
                        nc.sync.dma_start(
                            out=mt[:],
                            in_=maskT[nkb * KB:(nkb + 1) * KB,
                                      qb * QB:(qb + 1) * QB],
                        )
                        nc.vector.scalar_tensor_tensor(
                            out=sd[:, soff:soff + QB],
                            in0=mt[:],
                            scalar=1.0 / SCALE,
                            in1=sd[:, soff:soff + QB],
                            op0=mybir.AluOpType.mult,
                            op1=mybir.AluOpType.add,
                        )
                pt = ptpool.tile([128, 2 * QB], F16, tag="pt", name=f"pt_{i}")
                # one exp per chunk; the gap between diag members holds
                # garbage psum that is never consumed downstream
                nc.scalar.activation(
                    out=pt[:, :exp_end], in_=sd[:, :exp_end],
                    func=mybir.ActivationFunctionType.Exp,
                    scale=SCALE,
                )
                return pt

            def emit_back(ch, pt):
                qb, h, mem, first, last = ch
                if first:
                    o_psum[qb, h] = o_ps.tile([128, QB], F32, tag="o",
                                              name=f"o_{qb}_{h}")
                    pacc[qb, h] = paccp.tile([128, QB], F16, tag="pacc",
                                             name=f"pa_{qb}_{h}")
                oacc = o_psum[qb, h]
                pa = pacc[qb, h]
                for mi, (nkb, qoff, soff) in enumerate(mem):
                    w = QB - qoff
                    nc.tensor.matmul(
                        oacc[:, qoff:],
                        lhsT=v_sb[h][:, nkb * KB:(nkb + 1) * KB],
                        rhs=pt[:, soff:soff + w],
                        start=(first and mi == 0),
                        stop=(last and mi == len(mem) - 1),
                    )
                    # exp-tile accumulation for the rowsum: SBUF-only, so it
                    # lives on gpsimd and costs the DVE nothing
                    if first and mi == 0:
                        nc.gpsimd.tensor_copy(out=pa[:], in_=pt[:, :QB])
                    else:
                        nc.gpsimd.tensor_add(
                            pa[:, qoff:], pa[:, qoff:], pt[:, soff:soff + w])
                if last:
                    rps = y_ps.tile([128, QB], F32, tag="y", name=f"r_{qb}_{h}")
                    nc.tensor.matmul(
                        rps[:], lhsT=ones[:], rhs=pacc.pop((qb, h))[:],
                        start=True, stop=True,
                    )
                    rinv = small.tile([128, QB], F32, tag="rinv",
                                      name=f"ri_{qb}_{h}")
                    nc.vector.reciprocal_approx_fast(out=rinv[:], in_=rps[:])
                    ot = otpool.tile([128, QB], F16, tag="ot",
                                     name=f"ot_{qb}_{h}")
                    nc.vector.tensor_mul(ot[:], o_psum.pop((qb, h))[:], rinv[:])
                    oT_sb[qb, h] = ot
                    if h == HPC - 1:
                        ready.append(qb)

            def emit_qs_group(qb, qs):
                ysb = ypool.tile([128, D], F16, tag="ysb",
                                 name=f"ys_{qb}_{qs}")
                for dc in range(D // QB):
                    yps = y_ps.tile([128, QB], F32, tag="y",
                                    name=f"y_{qb}_{qs}_{dc}")
                    for h in range(HPC):
                        nc.tensor.matmul(
                            yps[:],
                            lhsT=oT_sb[qb, h][:, qs * 128:(qs + 1) * 128],
                            rhs=wo_sb[:, h * D + dc * QB: h * D + (dc + 1) * QB],
                            start=(h == 0),
                            stop=(h == HPC - 1),
                        )
                    # psum -> fp16 evacuation, mostly DVE with ACT taking a
                    # quarter to keep the DVE off the critical path
                    if dc == 3:
                        nc.scalar.copy(
                            out=ysb[:, dc * QB:(dc + 1) * QB], in_=yps[:])
                    else:
                        nc.vector.tensor_copy(
                            out=ysb[:, dc * QB:(dc + 1) * QB], in_=yps[:])
                row0 = qb * QB + qs * 128
                nc.sync.dma_start(out=y[row0:row0 + 128, :], in_=ysb[:])

            SKEW = 2        # chunks of scores/exp lookahead ahead of PV
            DELAY = 5       # chunks between normalize and outproj start
            for i, ch in enumerate(chunks):
                pts[i] = emit_front(i, ch)
                if i >= SKEW:
                    n_ready = len(ready)
                    emit_back(chunks[i - SKEW], pts.pop(i - SKEW))
                    if len(ready) > n_ready:
                        ready_at[ready[-1]] = i
                if fifo:
                    emit_qs_group(*fifo.pop(0))
                while ready and i - ready_at[ready[0]] >= DELAY:
                    qb0 = ready.pop(0)
                    fifo.extend((qb0, qs) for qs in range(QB // 128))
            for j, ch in enumerate(chunks[-SKEW:]):
                emit_back(ch, pts.pop(len(chunks) - SKEW + j))
                if fifo:
                    emit_qs_group(*fifo.pop(0))
            # keep the PE (and its HAM clock gate) busy while the final
            # head's rowsum-reciprocal chain resolves
            warm2 = y_ps.tile([128, QB], F32, tag="y", name="warm2")
            for wi in range(6):
                nc.tensor.matmul(
                    warm2[:], lhsT=ones[:], rhs=warm_sb[:],
                    start=(wi == 0), stop=(wi == 5),
                )
            for qb0 in ready:
                fifo.extend((qb0, qs) for qs in range(QB // 128))
            while fifo:
                emit_qs_group(*fifo.pop(0))

    nc.compile()
    return nc


_NC_CACHE: dict = {}


def _get_nc(causal: bool) -> bass.Bass:
    if causal not in _NC_CACHE:
        _NC_CACHE[causal] = build_nc(causal)
    return _NC_CACHE[causal]


def _make_in_maps(x, attn_mask, Wq, bq, Wk, bk, Wv, bv, Wo, bo, causal):
    xT = np.ascontiguousarray(x.T).astype(np.float16)
    maskT = None if causal else np.ascontiguousarray(attn_mask.T)
    in_maps = []
    for c in range(NCORES):
        sl = slice(c * CD, (c + 1) * CD)
        m = {
            "xT": xT,
            "wqT": np.ascontiguousarray(Wq[sl, :].T).astype(np.float16),
            "wkT": np.ascontiguousarray(Wk[sl, :].T).astype(np.float16),
            "wvT": np.ascontiguousarray(Wv[sl, :].T).astype(np.float16),
            "woT": np.ascontiguousarray(Wo[:, sl].T).astype(np.float16),
            "bq": np.ascontiguousarray(bq[sl]),
            "bk": np.ascontiguousarray(bk[sl]),
        }
        if maskT is not None:
            m["maskT"] = maskT
        in_maps.append(m)
    return in_maps


def _is_causal(attn_mask) -> bool:
    if attn_mask.shape != (N, N):
        return False
    expected = np.where(
        np.tril(np.ones((N, N), dtype=bool)), np.float32(0.0), np.float32(NEG)
    )
    return bool(np.array_equal(attn_mask, expected))


def run_spmd(in_maps, causal, **kwargs):
    nc = _get_nc(causal)
    return run_bass_kernel_spmd(nc, in_maps, core_ids=list(range(NCORES)), **kwargs)


def kernel(x, attn_mask, Wq, bq, Wk, bk, Wv, bv, Wo, bo):
    causal = _is_causal(np.asarray(attn_mask))
    in_maps = _make_in_maps(
        np.asarray(x, np.float32), np.asarray(attn_mask, np.float32),
        np.asarray(Wq, np.float32), np.asarray(bq, np.float32),
        np.asarray(Wk, np.float32), np.asarray(bk, np.float32),
        np.asarray(Wv, np.float32), np.asarray(bv, np.float32),
        np.asarray(Wo, np.float32), np.asarray(bo, np.float32),
        causal,
    )
    res = run_spmd(in_maps, causal)
    out = np.zeros((N, D), np.float32)
    for r in res.results:
        out += r["y"]
    # v bias folds into the output bias exactly (attn rows sum to 1); both
    # are added once here instead of per-core on device
    out += (np.asarray(bo, np.float32) + np.asarray(Wo, np.float32)
            @ np.asarray(bv, np.float32))[None, :]
    return out


# revision 13
# speedup vs baseline: 1.2151x; 1.2151x over previous
"""Causal self-attention TRN2 kernel, tensor-parallel over heads on 8 NeuronCores.

Model (N=4096 tokens, D=2048, H=16 heads, HD=128):
    q = x @ Wq.T + bq ; k = x @ Wk.T + bk ; v = x @ Wv.T + bv   (per head)
    attn = softmax(q k^T / sqrt(HD) + causal_mask)
    y = concat_h(attn @ v) @ Wo.T + bo
Sharding: core c owns heads {2c, 2c+1}; host sums the 8 partial outputs.

Per-core layout (same as the v1 kernel): x fed transposed, qT/kT produced
(HD x N) fp16, scores computed transposed sT[k,q] so PV needs no transposes,
v stored (n, hd) via PE transposes, v-bias folded into the output bias.

v2 performance changes:
  * Phase 1 runs the 16-chunk contraction per OUTPUT (q0,q1,k0,k1,v0,v1)
    instead of per chunk, so each output's PSUM bank frees ~17us before its
    next use -- removes the per-stripe psum-reuse bubble.  k biases are
    applied on DVE (per-partition scalar) in parallel with q's ACT path.
  * The softmax rowsum no longer streams every exp tile through the PE
    (was ~14%% of PE work): exp tiles are accumulated over key blocks on
    DVE (head 0) / gpsimd (head 1) into a fp16 pacc tile, and a single
    512-col ones-matmul per (qb, head) contracts the 128 lanes.
  * Scores are computed into [128,1024] 2-bank PSUM tiles (two key blocks
    per tile) and exp'd with ONE ACT op per pair -- amortizes ACT fixed
    overhead; ACT would otherwise pace the attention once the rowsum
    matmuls are gone.  Diagonal blocks share paired tiles too, with
    precomputed combined triangular masks (one DVE add per 2 blocks).
  * 1/rowsum via reciprocal_approx_fast (~5x faster than reciprocal,
    ~18 correct bits -- plenty for a softmax denominator).
  * y is written fp16 (host accumulates in fp32), one DMA per 128-row slab.
  * Output-projection bias adds alternate DVE/gpsimd; outproj is emitted
    one 128-row group per pipeline slot to avoid 7us PE bursts starving ACT.
"""

from contextlib import ExitStack

import numpy as np

import concourse.bass as bass
import concourse.tile as tile
from concourse import bacc
from concourse import mybir
from concourse.bass_utils import run_bass_kernel_spmd
from concourse.masks import make_identity

N, D, H, HD = 4096, 2048, 16, 128
NCORES = 8
HPC = H // NCORES            # heads per core (2)
CD = HPC * HD                # per-core head-dim slice (256)
SCALE = 1.0 / float(np.sqrt(HD))
NEG = -1e9

QB = 512                     # query block (free dim of moving operands)
KB = 128                     # key block (partition dim of scores)
NQB = N // QB                # 8
KC = D // 128                # contraction chunks for projections (16)

F32 = mybir.dt.float32
F16 = mybir.dt.float16


def build_nc(causal: bool = True) -> bass.Bass:
    nc = bacc.Bacc(None)

    xT = nc.declare_dram_parameter("xT", [D, N], F16, isOutput=False)
    wqT = nc.declare_dram_parameter("wqT", [D, CD], F16, isOutput=False)
    wkT = nc.declare_dram_parameter("wkT", [D, CD], F16, isOutput=False)
    wvT = nc.declare_dram_parameter("wvT", [D, CD], F16, isOutput=False)
    woT = nc.declare_dram_parameter("woT", [CD, D], F16, isOutput=False)
    bq = nc.declare_dram_parameter("bq", [CD], F32, isOutput=False)
    bk = nc.declare_dram_parameter("bk", [CD], F32, isOutput=False)
    maskT = None
    if not causal:
        maskT = nc.declare_dram_parameter("maskT", [N, N], F32, isOutput=False)
    y = nc.declare_dram_parameter("y", [N, D], F16, isOutput=True)

    with tile.TileContext(nc) as tc, ExitStack() as ctx:
        persist = ctx.enter_context(tc.tile_pool(name="persist", bufs=1))

        # -------- setup: phase-1-critical DMAs first ----------------------
        bq_sb = persist.tile([128, HPC], F32, tag="bq")
        bk_sb = persist.tile([128, HPC], F32, tag="bk")
        nc.sync.dma_start(out=bq_sb[:], in_=bq[:].rearrange("(h p) -> p h", p=128))
        nc.sync.dma_start(out=bk_sb[:], in_=bk[:].rearrange("(h p) -> p h", p=128))

        wproj = ctx.enter_context(tc.tile_pool(name="wproj", bufs=1))
        wq_sb = wproj.tile([128, KC * CD], F16, tag="wq")
        wk_sb = wproj.tile([128, KC * CD], F16, tag="wk")
        wv_sb = wproj.tile([128, KC * CD], F16, tag="wv")
        for w_sb, w_dram in ((wq_sb, wqT), (wk_sb, wkT), (wv_sb, wvT)):
            for c in range(KC):
                nc.sync.dma_start(
                    out=w_sb[:, c * CD:(c + 1) * CD],
                    in_=w_dram[c * 128:(c + 1) * 128, :],
                )

        # phase-2 weights; transfers overlap phase-1 compute.  The output
        # bias is added on the HOST after summing the 8 partials, so the
        # outproj evacuation is a plain psum->fp16 copy.
        wo_sb = persist.tile([128, HPC * D], F16, tag="wo")
        nc.sync.dma_start(
            out=wo_sb[:].rearrange("p (h d) -> p h d", h=HPC),
            in_=woT[:].rearrange("(h p) d -> p h d", p=128),
        )

        ident = persist.tile([128, 128], F16, tag="ident")
        # ones stationary: rowsum matmul broadcasts the lane-sum to all
        # 128 output partitions at no extra cost
        ones = persist.tile([128, 128], F16, tag="ones")
        nc.vector.memset(ones[:], 1.0)
        # combined triangular masks for the paired diagonal blocks:
        # m[k, seg*512 + t] = 0 if t >= k else NEG for each member segment
        m1 = m2 = None
        if causal:
            m1 = persist.tile([128, QB + 384], F32, tag="m1")   # widths 512,384
            m2 = persist.tile([128, QB + 128], F32, tag="m2")   # widths 256,128
            nc.vector.memset(m1[:], 0.0)
            nc.vector.memset(m2[:], 0.0)

        def emit_setup_selects():
            # gpsimd setup ops, emitted after the first stripe's x DMAs so
            # they don't block the gpsimd queue head
            make_identity(nc, ident[:])
            if causal:
                for mt, spans in ((m1, (QB, 384)), (m2, (256, 128))):
                    for seg, w in enumerate(spans):
                        nc.gpsimd.affine_select(
                            out=mt[:, seg * QB:seg * QB + w],
                            in_=mt[:, seg * QB:seg * QB + w],
                            compare_op=mybir.AluOpType.is_ge,
                            fill=NEG,
                            base=0,
                            pattern=[[1, w]],
                            channel_multiplier=-1,
                        )

        # PE warm-up: dependency-free matmuls fill the DMA-startup window
        # and push the HAM clock gate to full rate before real work
        warm_sb = persist.tile([128, QB], F16, tag="warm")
        nc.vector.memset(warm_sb[:], 0.0)

        # Persistent activations: qT/kT per head (HD x N) fp16; v per head
        # stored (128, 32*128) with free = (n_block, hd) i.e. (N x HD) layout.
        qT = [persist.tile([128, N], F16, tag=f"qT{h}", name=f"qT{h}")
              for h in range(HPC)]
        kT = [persist.tile([128, N], F16, tag=f"kT{h}", name=f"kT{h}")
              for h in range(HPC)]
        v_sb = [persist.tile([128, N], F16, tag=f"v{h}", name=f"v{h}")
                for h in range(HPC)]

        # ---------------- phase 1: QKV projections ----------------------
        xin = ctx.enter_context(tc.tile_pool(name="xin", bufs=24))
        vtpool = ctx.enter_context(tc.tile_pool(name="vt", bufs=3))
        with ExitStack() as p1ps:
            qkv_ps = p1ps.enter_context(
                tc.tile_pool(name="qkv_ps", bufs=6, space="PSUM"))
            tp_ps = p1ps.enter_context(
                tc.tile_pool(name="tp_ps", bufs=2, space="PSUM"))

            warm_ps = qkv_ps.tile([128, QB], F32, tag="qkv", name="warm_ps")
            for wi in range(20):
                nc.tensor.matmul(
                    warm_ps[:], lhsT=ones[:], rhs=warm_sb[:],
                    start=(wi == 0), stop=(wi == 19),
                )

            # v-transposes are deferred one output-group so the PE never
            # waits on the DVE psum->sbuf copy feeding them
            deferred = []

            def flush_deferred():
                while deferred:
                    deferred.pop(0)()

            W = {"q": wq_sb, "k": wk_sb, "v": wv_sb}
            for nb in range(NQB):
                xts = []
                for c in range(KC):
                    xt = xin.tile([128, QB], F16, tag="xt", name=f"xt_{nb}_{c}")
                    nc.gpsimd.dma_start(
                        out=xt[:],
                        in_=xT[c * 128:(c + 1) * 128, nb * QB:(nb + 1) * QB],
                    )
                    xts.append(xt)
                if nb == 0:
                    emit_setup_selects()
                outs = [("q", 0), ("q", 1), ("k", 0), ("k", 1),
                        ("v", 0), ("v", 1)]
                if nb == NQB - 1:
                    # last stripe: v first so its deferred transposes flush
                    # under the q/k matmuls instead of stalling at phase end
                    outs = outs[4:] + outs[:4]
                for nm, h in outs:
                    ps = qkv_ps.tile([128, QB], F32, tag="qkv",
                                     name=f"ps_{nm}{h}_{nb}")
                    w_sb = W[nm]
                    for c in range(KC):
                        nc.tensor.matmul(
                            ps[:],
                            lhsT=w_sb[:, c * CD + h * HD: c * CD + (h + 1) * HD],
                            rhs=xts[c][:],
                            start=(c == 0),
                            stop=(c == KC - 1),
                        )
                    flush_deferred()
                    if nm == "q":
                        nc.scalar.activation(
                            out=qT[h][:, nb * QB:(nb + 1) * QB], in_=ps[:],
                            func=mybir.ActivationFunctionType.Identity,
                            bias=bq_sb[:, h:h + 1], scale=1.0,
                        )
                    elif nm == "k":
                        nc.vector.tensor_scalar_add(
                            out=kT[h][:, nb * QB:(nb + 1) * QB], in0=ps[:],
                            scalar1=bk_sb[:, h:h + 1],
                        )
                    else:
                        vt = vtpool.tile([128, QB], F16, tag="vt",
                                         name=f"vt_{nb}_{h}")
                        nc.vector.tensor_copy(out=vt[:], in_=ps[:])

                        def mk(vt=vt, nb=nb, h=h):
                            def go():
                                for s in range(QB // 128):
                                    tp = tp_ps.tile([128, 128], F16, tag="tp",
                                                    name=f"tp_{nb}_{h}_{s}")
                                    nc.tensor.transpose(
                                        tp[:], vt[:, s * 128:(s + 1) * 128],
                                        ident[:],
                                    )
                                    nblk = nb * (QB // 128) + s
                                    nc.vector.tensor_copy(
                                        out=v_sb[h][:, nblk * 128:(nblk + 1) * 128],
                                        in_=tp[:],
                                    )
                            return go
                        deferred.append(mk())
                if nb == NQB - 1:
                    flush_deferred()

        # ---------------- phase 2: attention + output projection --------
        with ExitStack() as p2:
            ptpool = p2.enter_context(tc.tile_pool(name="pt", bufs=4))
            paccp = p2.enter_context(tc.tile_pool(name="pacc", bufs=4))
            otpool = p2.enter_context(tc.tile_pool(name="ot", bufs=4))
            ypool = p2.enter_context(tc.tile_pool(name="yout", bufs=4))
            small = p2.enter_context(tc.tile_pool(name="small", bufs=4))
            mtpool = p2.enter_context(tc.tile_pool(name="mt", bufs=6))
            s_ps = p2.enter_context(
                tc.tile_pool(name="s_ps", bufs=2, space="PSUM"))   # 2x2 banks
            o_ps = p2.enter_context(
                tc.tile_pool(name="o_ps", bufs=2, space="PSUM"))   # 2 banks
            y_ps = p2.enter_context(
                tc.tile_pool(name="y_ps", bufs=2, space="PSUM"))   # 2 banks

            # chunk = up to two key blocks sharing one [128,1024] score tile
            # and ONE exp.  members: (nkb, qoff, sd_off); qoff>0 only on
            # diagonal-straddling blocks (columns below qoff fully masked).
            chunks = []
            for qb in range(NQB):
                nfull = 4 * qb if causal else N // KB
                for h in range(HPC):
                    mem_chunks = []
                    for j in range(0, nfull - 1, 2):
                        mem_chunks.append([(j, 0, 0), (j + 1, 0, QB)])
                    if causal:
                        dg = 4 * qb
                        mem_chunks.append([(dg, 0, 0), (dg + 1, KB, QB)])
                        mem_chunks.append([(dg + 2, 2 * KB, 0),
                                           (dg + 3, 3 * KB, QB)])
                    for ci, mem in enumerate(mem_chunks):
                        chunks.append((qb, h, mem, ci == 0,
                                       ci == len(mem_chunks) - 1))

            pts = {}
            o_psum = {}
            pacc = {}
            oT_sb = {}
            ready = []
            ready_at = {}
            fifo = []

            def emit_front(i, ch):
                qb, h, mem, first, last = ch
                sd = s_ps.tile([128, 2 * QB], F32, tag="s", name=f"s_{i}")
                for nkb, qoff, soff in mem:
                    w = QB - qoff
                    nc.tensor.matmul(
                        sd[:, soff:soff + w],
                        lhsT=kT[h][:, nkb * KB:(nkb + 1) * KB],
                        rhs=qT[h][:, qb * QB + qoff:(qb + 1) * QB],
                        start=True,
                        stop=True,
                    )
                exp_end = mem[-1][2] + (QB - mem[-1][1])
                if causal and mem[0][0] >= 4 * qb:
                    # paired diagonal blocks: one combined-mask add
                    # (PSUM access -> must be DVE, gpsimd cannot touch PSUM)
                    mt = m1 if mem[0][1] == 0 else m2
                    nc.vector.tensor_add(
                        sd[:, :exp_end], sd[:, :exp_end], mt[:, :exp_end])
                elif not causal:
                    for nkb, qoff, soff in mem:
                        mt = mtpool.tile([128, QB], F32, tag="mt",
                                         name=f"mt_{i}_{soff}")
                        nc.sync.dma_start(
                            out=mt[:],
                            in_=maskT[nkb * KB:(nkb + 1) * KB,
                                      qb * QB:(qb + 1) * QB],
                        )
                        nc.vector.scalar_tensor_tensor(
                            out=sd[:, soff:soff + QB],
                            in0=mt[:],
                            scalar=1.0 / SCALE,
                            in1=sd[:, soff:soff + QB],
                            op0=mybir.AluOpType.mult,
                            op1=mybir.AluOpType.add,
                        )
                pt = ptpool.tile([128, 2 * QB], F16, tag="pt", name=f"pt_{i}")
                # one exp per chunk; the gap between diag members holds
                # garbage psum that is never consumed downstream
                nc.scalar.activation(
                    out=pt[:, :exp_end], in_=sd[:, :exp_end],
                    func=mybir.ActivationFunctionType.Exp,
                    scale=SCALE,
                )
                return pt

            def emit_back(ch, pt):
                qb, h, mem, first, last = ch
                if first:
                    o_psum[qb, h] = o_ps.tile([128, QB], F32, tag="o",
                                              name=f"o_{qb}_{h}")
                    pacc[qb, h] = paccp.tile([128, QB], F16, tag="pacc",
                                             name=f"pa_{qb}_{h}")
                oacc = o_psum[qb, h]
                pa = pacc[qb, h]
                for mi, (nkb, qoff, soff) in enumerate(mem):
                    w = QB - qoff
                    nc.tensor.matmul(
                        oacc[:, qoff:],
                        lhsT=v_sb[h][:, nkb * KB:(nkb + 1) * KB],
                        rhs=pt[:, soff:soff + w],
                        start=(first and mi == 0),
                        stop=(last and mi == len(mem) - 1),
                    )
                # exp-tile accumulation for the rowsum, split across engines:
                # gpsimd (slow but idle) folds member 1 into member 0's
                # columns inside the pt tile (SBUF-only), then ONE fast DVE
                # add merges the chunk into pacc.
                (n0, q0, _), (n1, q1, _) = mem
                w1 = QB - q1
                nc.gpsimd.tensor_add(
                    pt[:, q1 - q0:q1 - q0 + w1],
                    pt[:, q1 - q0:q1 - q0 + w1],
                    pt[:, QB:QB + w1],
                )
                w0 = QB - q0
                if first:
                    nc.vector.tensor_copy(out=pa[:], in_=pt[:, :QB])
                else:
                    nc.vector.tensor_add(
                        pa[:, q0:], pa[:, q0:], pt[:, :w0])
                if last:
                    finals.append((qb, h))

            def emit_final(qb, h):
                rps = y_ps.tile([128, QB], F32, tag="y", name=f"r_{qb}_{h}")
                nc.tensor.matmul(
                    rps[:], lhsT=ones[:], rhs=pacc.pop((qb, h))[:],
                    start=True, stop=True,
                )
                rinv = small.tile([128, QB], F32, tag="rinv",
                                  name=f"ri_{qb}_{h}")
                nc.vector.reciprocal_approx_fast(out=rinv[:], in_=rps[:])
                ot = otpool.tile([128, QB], F16, tag="ot",
                                 name=f"ot_{qb}_{h}")
                nc.vector.tensor_mul(ot[:], o_psum.pop((qb, h))[:], rinv[:])
                oT_sb[qb, h] = ot
                if h == HPC - 1:
                    ready.append(qb)
                    return True
                return False

            def emit_qs_group(qb, qs):
                ysb = ypool.tile([128, D], F16, tag="ysb",
                                 name=f"ys_{qb}_{qs}")
                for dc in range(D // QB):
                    yps = y_ps.tile([128, QB], F32, tag="y",
                                    name=f"y_{qb}_{qs}_{dc}")
                    for h in range(HPC):
                        nc.tensor.matmul(
                            yps[:],
                            lhsT=oT_sb[qb, h][:, qs * 128:(qs + 1) * 128],
                            rhs=wo_sb[:, h * D + dc * QB: h * D + (dc + 1) * QB],
                            start=(h == 0),
                            stop=(h == HPC - 1),
                        )
                    # psum -> fp16 evacuation, split DVE/ACT to keep the
                    # DVE off the critical path
                    if dc >= 2:
                        nc.scalar.copy(
                            out=ysb[:, dc * QB:(dc + 1) * QB], in_=yps[:])
                    else:
                        nc.vector.tensor_copy(
                            out=ysb[:, dc * QB:(dc + 1) * QB], in_=yps[:])
                row0 = qb * QB + qs * 128
                nc.sync.dma_start(out=y[row0:row0 + 128, :], in_=ysb[:])

            SKEW = 2        # chunks of scores/exp lookahead ahead of PV
            DELAY = 5       # chunks between normalize and outproj start
            finals = []     # (qb, h) pairs whose pacc chain is fully
                            # emitted; their rowsum matmul is emitted one
                            # chunk later so the PE never waits on the
                            # gpsimd+DVE combine latency
            pending = []    # (qb, h, pushed_at)
            for i, ch in enumerate(chunks):
                pts[i] = emit_front(i, ch)
                if i >= SKEW:
                    emit_back(chunks[i - SKEW], pts.pop(i - SKEW))
                    while finals:
                        pending.append(finals.pop(0) + (i,))
                while pending and pending[0][2] < i:
                    qb0, h0, _ = pending.pop(0)
                    if emit_final(qb0, h0):
                        ready_at[ready[-1]] = i
                if fifo:
                    emit_qs_group(*fifo.pop(0))
                while ready and i - ready_at[ready[0]] >= DELAY:
                    qb0 = ready.pop(0)
                    fifo.extend((qb0, qs) for qs in range(QB // 128))
            for j, ch in enumerate(chunks[-SKEW:]):
                emit_back(ch, pts.pop(len(chunks) - SKEW + j))
                if fifo:
                    emit_qs_group(*fifo.pop(0))
            for qb0, h0, _ in pending:
                emit_final(qb0, h0)
            while finals:
                emit_final(*finals.pop(0))
            # keep the PE (and its HAM clock gate) busy while the final
            # head's rowsum-reciprocal chain resolves
            warm2 = y_ps.tile([128, QB], F32, tag="y", name="warm2")
            for wi in range(6):
                nc.tensor.matmul(
                    warm2[:], lhsT=ones[:], rhs=warm_sb[:],
                    start=(wi == 0), stop=(wi == 5),
                )
            for qb0 in ready:
                fifo.extend((qb0, qs) for qs in range(QB // 128))
            while fifo:
                emit_qs_group(*fifo.pop(0))

    nc.compile()
    return nc


_NC_CACHE: dict = {}


def _get_nc(causal: bool) -> bass.Bass:
    if causal not in _NC_CACHE:
        _NC_CACHE[causal] = build_nc(causal)
    return _NC_CACHE[causal]


def _make_in_maps(x, attn_mask, Wq, bq, Wk, bk, Wv, bv, Wo, bo, causal):
    xT = np.ascontiguousarray(x.T).astype(np.float16)
    maskT = None if causal else np.ascontiguousarray(attn_mask.T)
    in_maps = []
    for c in range(NCORES):
        sl = slice(c * CD, (c + 1) * CD)
        m = {
            "xT": xT,
            "wqT": np.ascontiguousarray(Wq[sl, :].T).astype(np.float16),
            "wkT": np.ascontiguousarray(Wk[sl, :].T).astype(np.float16),
            "wvT": np.ascontiguousarray(Wv[sl, :].T).astype(np.float16),
            "woT": np.ascontiguousarray(Wo[:, sl].T).astype(np.float16),
            "bq": np.ascontiguousarray(bq[sl]),
            "bk": np.ascontiguousarray(bk[sl]),
        }
        if maskT is not None:
            m["maskT"] = maskT
        in_maps.append(m)
    return in_maps


def _is_causal(attn_mask) -> bool:
    if attn_mask.shape != (N, N):
        return False
    expected = np.where(
        np.tril(np.ones((N, N), dtype=bool)), np.float32(0.0), np.float32(NEG)
    )
    return bool(np.array_equal(attn_mask, expected))


def run_spmd(in_maps, causal, **kwargs):
    nc = _get_nc(causal)
    return run_bass_kernel_spmd(nc, in_maps, core_ids=list(range(NCORES)), **kwargs)


def kernel(x, attn_mask, Wq, bq, Wk, bk, Wv, bv, Wo, bo):
    causal = _is_causal(np.asarray(attn_mask))
    in_maps = _make_in_maps(
        np.asarray(x, np.float32), np.asarray(attn_mask, np.float32),
        np.asarray(Wq, np.float32), np.asarray(bq, np.float32),
        np.asarray(Wk, np.float32), np.asarray(bk, np.float32),
        np.asarray(Wv, np.float32), np.asarray(bv, np.float32),
        np.asarray(Wo, np.float32), np.asarray(bo, np.float32),
        causal,
    )
    res = run_spmd(in_maps, causal)
    out = np.zeros((N, D), np.float32)
    for r in res.results:
        out += r["y"]
    # v bias folds into the output bias exactly (attn rows sum to 1); both
    # are added once here instead of per-core on device
    out += (np.asarray(bo, np.float32) + np.asarray(Wo, np.float32)
            @ np.asarray(bv, np.float32))[None, :]
    return out


# revision 19
# speedup vs baseline: 1.3037x; 1.0729x over previous
"""Causal self-attention TRN2 kernel, tensor-parallel over heads on 8 NeuronCores.

Model (N=4096 tokens, D=2048, H=16 heads, HD=128):
    q = x @ Wq.T + bq ; k = x @ Wk.T + bk ; v = x @ Wv.T + bv   (per head)
    attn = softmax(q k^T / sqrt(HD) + causal_mask)
    y = concat_h(attn @ v) @ Wo.T + bo
Sharding: core c owns heads {2c, 2c+1}; host sums the 8 partial outputs.

Per-core layout (same as the v1 kernel): x fed transposed, qT/kT produced
(HD x N) fp16, scores computed transposed sT[k,q] so PV needs no transposes,
v stored (n, hd) via PE transposes, v-bias folded into the output bias.

v2 performance changes:
  * Phase 1 runs the 16-chunk contraction per OUTPUT (q0,q1,k0,k1,v0,v1)
    instead of per chunk, so each output's PSUM bank frees ~17us before its
    next use -- removes the per-stripe psum-reuse bubble.  k biases are
    applied on DVE (per-partition scalar) in parallel with q's ACT path.
  * The softmax rowsum no longer streams every exp tile through the PE
    (was ~14%% of PE work): exp tiles are accumulated over key blocks on
    DVE (head 0) / gpsimd (head 1) into a fp16 pacc tile, and a single
    512-col ones-matmul per (qb, head) contracts the 128 lanes.
  * Scores are computed into [128,1024] 2-bank PSUM tiles (two key blocks
    per tile) and exp'd with ONE ACT op per pair -- amortizes ACT fixed
    overhead; ACT would otherwise pace the attention once the rowsum
    matmuls are gone.  Diagonal blocks share paired tiles too, with
    precomputed combined triangular masks (one DVE add per 2 blocks).
  * 1/rowsum via reciprocal_approx_fast (~5x faster than reciprocal,
    ~18 correct bits -- plenty for a softmax denominator).
  * y is written fp16 (host accumulates in fp32), one DMA per 128-row slab.
  * Output-projection bias adds alternate DVE/gpsimd; outproj is emitted
    one 128-row group per pipeline slot to avoid 7us PE bursts starving ACT.
"""

from contextlib import ExitStack

import numpy as np

import concourse.bass as bass
import concourse.tile as tile
from concourse import bacc
from concourse import mybir
from concourse.bass_utils import run_bass_kernel_spmd
from concourse.masks import make_identity

N, D, H, HD = 4096, 2048, 16, 128
NCORES = 8
HPC = H // NCORES            # heads per core (2)
CD = HPC * HD                # per-core head-dim slice (256)
SCALE = 1.0 / float(np.sqrt(HD))
NEG = -1e9

QB = 512                     # query block (free dim of moving operands)
KB = 128                     # key block (partition dim of scores)
NQB = N // QB                # 8
KC = D // 128                # contraction chunks for projections (16)

F32 = mybir.dt.float32
F16 = mybir.dt.float16


def build_nc(causal: bool = True) -> bass.Bass:
    nc = bacc.Bacc(None)

    xT = nc.declare_dram_parameter("xT", [D, N], F16, isOutput=False)
    wqT = nc.declare_dram_parameter("wqT", [D, CD], F16, isOutput=False)
    wkT = nc.declare_dram_parameter("wkT", [D, CD], F16, isOutput=False)
    wvT = nc.declare_dram_parameter("wvT", [D, CD], F16, isOutput=False)
    woT = nc.declare_dram_parameter("woT", [CD, D], F16, isOutput=False)
    bq = nc.declare_dram_parameter("bq", [CD], F32, isOutput=False)
    bk = nc.declare_dram_parameter("bk", [CD], F32, isOutput=False)
    maskT = None
    if not causal:
        maskT = nc.declare_dram_parameter("maskT", [N, N], F32, isOutput=False)
    y = nc.declare_dram_parameter("y", [N, D], F16, isOutput=True)

    with tile.TileContext(nc) as tc, ExitStack() as ctx:
        persist = ctx.enter_context(tc.tile_pool(name="persist", bufs=1))

        # -------- setup: phase-1-critical DMAs first ----------------------
        bq_sb = persist.tile([128, HPC], F32, tag="bq")
        bk_sb = persist.tile([128, HPC], F32, tag="bk")
        nc.sync.dma_start(out=bq_sb[:], in_=bq[:].rearrange("(h p) -> p h", p=128))
        nc.sync.dma_start(out=bk_sb[:], in_=bk[:].rearrange("(h p) -> p h", p=128))

        wproj = ctx.enter_context(tc.tile_pool(name="wproj", bufs=1))
        wq_sb = wproj.tile([128, KC * CD], F16, tag="wq")
        wk_sb = wproj.tile([128, KC * CD], F16, tag="wk")
        wv_sb = wproj.tile([128, KC * CD], F16, tag="wv")
        for w_sb, w_dram in ((wq_sb, wqT), (wk_sb, wkT), (wv_sb, wvT)):
            for c in range(KC):
                nc.sync.dma_start(
                    out=w_sb[:, c * CD:(c + 1) * CD],
                    in_=w_dram[c * 128:(c + 1) * 128, :],
                )

        # phase-2 weights; transfers overlap phase-1 compute.  The output
        # bias is added on the HOST after summing the 8 partials, so the
        # outproj evacuation is a plain psum->fp16 copy.
        wo_sb = persist.tile([128, HPC * D], F16, tag="wo")
        nc.sync.dma_start(
            out=wo_sb[:].rearrange("p (h d) -> p h d", h=HPC),
            in_=woT[:].rearrange("(h p) d -> p h d", p=128),
        )

        ident = persist.tile([128, 128], F16, tag="ident")
        # ones stationary: rowsum matmul broadcasts the lane-sum to all
        # 128 output partitions at no extra cost
        ones = persist.tile([128, 128], F16, tag="ones")
        nc.vector.memset(ones[:], 1.0)
        # combined triangular masks for the paired diagonal blocks:
        # m[k, seg*512 + t] = 0 if t >= k else NEG for each member segment
        m1 = m2 = None
        if causal:
            m1 = persist.tile([128, QB + 384], F32, tag="m1")   # widths 512,384
            m2 = persist.tile([128, QB + 128], F32, tag="m2")   # widths 256,128
            nc.vector.memset(m1[:], 0.0)
            nc.vector.memset(m2[:], 0.0)

        def emit_setup_selects():
            # gpsimd setup ops, emitted after the first stripe's x DMAs so
            # they don't block the gpsimd queue head
            make_identity(nc, ident[:])
            if causal:
                for mt, spans in ((m1, (QB, 384)), (m2, (256, 128))):
                    for seg, w in enumerate(spans):
                        nc.gpsimd.affine_select(
                            out=mt[:, seg * QB:seg * QB + w],
                            in_=mt[:, seg * QB:seg * QB + w],
                            compare_op=mybir.AluOpType.is_ge,
                            fill=NEG,
                            base=0,
                            pattern=[[1, w]],
                            channel_multiplier=-1,
                        )

        # PE warm-up: dependency-free matmuls fill the DMA-startup window
        # and push the HAM clock gate to full rate before real work
        warm_sb = persist.tile([128, QB], F16, tag="warm")
        nc.vector.memset(warm_sb[:], 0.0)

        # Persistent activations: qT/kT per head (HD x N) fp16; v per head
        # stored (128, 32*128) with free = (n_block, hd) i.e. (N x HD) layout.
        qT = [persist.tile([128, N], F16, tag=f"qT{h}", name=f"qT{h}")
              for h in range(HPC)]
        kT = [persist.tile([128, N], F16, tag=f"kT{h}", name=f"kT{h}")
              for h in range(HPC)]
        v_sb = [persist.tile([128, N], F16, tag=f"v{h}", name=f"v{h}")
                for h in range(HPC)]

        # ---------------- phase 1: QKV projections ----------------------
        xin = ctx.enter_context(tc.tile_pool(name="xin", bufs=32))
        vtpool = ctx.enter_context(tc.tile_pool(name="vt", bufs=3))
        with ExitStack() as p1ps:
            qkv_ps = p1ps.enter_context(
                tc.tile_pool(name="qkv_ps", bufs=6, space="PSUM"))
            tp_ps = p1ps.enter_context(
                tc.tile_pool(name="tp_ps", bufs=2, space="PSUM"))

            warm_ps = qkv_ps.tile([128, QB], F32, tag="qkv", name="warm_ps")
            for wi in range(20):
                nc.tensor.matmul(
                    warm_ps[:], lhsT=ones[:], rhs=warm_sb[:],
                    start=(wi == 0), stop=(wi == 19),
                )

            # v-transposes are deferred one output-group so the PE never
            # waits on the DVE psum->sbuf copy feeding them
            deferred = []

            def flush_deferred():
                while deferred:
                    deferred.pop(0)()

            W = {"q": wq_sb, "k": wk_sb, "v": wv_sb}
            for nb in range(NQB):
                xts = []
                for c in range(KC):
                    xt = xin.tile([128, QB], F16, tag="xt", name=f"xt_{nb}_{c}")
                    nc.gpsimd.dma_start(
                        out=xt[:],
                        in_=xT[c * 128:(c + 1) * 128, nb * QB:(nb + 1) * QB],
                    )
                    xts.append(xt)
                if nb == 0:
                    emit_setup_selects()
                outs = [("q", 0), ("q", 1), ("k", 0), ("k", 1),
                        ("v", 0), ("v", 1)]
                if nb == NQB - 1:
                    # last stripe: v first so its deferred transposes flush
                    # under the q/k matmuls instead of stalling at phase end
                    outs = outs[4:] + outs[:4]
                for nm, h in outs:
                    ps = qkv_ps.tile([128, QB], F32, tag="qkv",
                                     name=f"ps_{nm}{h}_{nb}")
                    w_sb = W[nm]
                    for c in range(KC):
                        nc.tensor.matmul(
                            ps[:],
                            lhsT=w_sb[:, c * CD + h * HD: c * CD + (h + 1) * HD],
                            rhs=xts[c][:],
                            start=(c == 0),
                            stop=(c == KC - 1),
                        )
                    flush_deferred()
                    if nm == "q":
                        nc.scalar.activation(
                            out=qT[h][:, nb * QB:(nb + 1) * QB], in_=ps[:],
                            func=mybir.ActivationFunctionType.Identity,
                            bias=bq_sb[:, h:h + 1], scale=1.0,
                        )
                    elif nm == "k":
                        nc.vector.tensor_scalar_add(
                            out=kT[h][:, nb * QB:(nb + 1) * QB], in0=ps[:],
                            scalar1=bk_sb[:, h:h + 1],
                        )
                    else:
                        vt = vtpool.tile([128, QB], F16, tag="vt",
                                         name=f"vt_{nb}_{h}")
                        nc.vector.tensor_copy(out=vt[:], in_=ps[:])

                        def mk(vt=vt, nb=nb, h=h):
                            def go():
                                for s in range(QB // 128):
                                    tp = tp_ps.tile([128, 128], F16, tag="tp",
                                                    name=f"tp_{nb}_{h}_{s}")
                                    nc.tensor.transpose(
                                        tp[:], vt[:, s * 128:(s + 1) * 128],
                                        ident[:],
                                    )
                                    nblk = nb * (QB // 128) + s
                                    nc.vector.tensor_copy(
                                        out=v_sb[h][:, nblk * 128:(nblk + 1) * 128],
                                        in_=tp[:],
                                    )
                            return go
                        deferred.append(mk())
                if nb == NQB - 1:
                    flush_deferred()

        # ---------------- phase 2: attention + output projection --------
        with ExitStack() as p2:
            ptpool = p2.enter_context(tc.tile_pool(name="pt", bufs=5))
            paccp = p2.enter_context(tc.tile_pool(name="pacc", bufs=4))
            otpool = p2.enter_context(tc.tile_pool(name="ot", bufs=6))
            ypool = p2.enter_context(tc.tile_pool(name="yout", bufs=4))
            small = p2.enter_context(tc.tile_pool(name="small", bufs=4))
            mtpool = p2.enter_context(tc.tile_pool(name="mt", bufs=6))
            s_ps = p2.enter_context(
                tc.tile_pool(name="s_ps", bufs=2, space="PSUM"))   # 2x2 banks
            o_ps = p2.enter_context(
                tc.tile_pool(name="o_ps", bufs=2, space="PSUM"))   # 2 banks
            y_ps = p2.enter_context(
                tc.tile_pool(name="y_ps", bufs=2, space="PSUM"))   # 2 banks

            # chunk = up to two key blocks sharing one [128,1024] score tile
            # and ONE exp.  members: (nkb, qoff, sd_off); qoff>0 only on
            # diagonal-straddling blocks (columns below qoff fully masked).
            chunks = []
            for qb in range(NQB):
                nfull = 4 * qb if causal else N // KB
                for h in range(HPC):
                    mem_chunks = []
                    for j in range(0, nfull - 1, 2):
                        mem_chunks.append([(j, 0, 0), (j + 1, 0, QB)])
                    if causal:
                        dg = 4 * qb
                        mem_chunks.append([(dg, 0, 0), (dg + 1, KB, QB)])
                        mem_chunks.append([(dg + 2, 2 * KB, 0),
                                           (dg + 3, 3 * KB, QB)])
                    for ci, mem in enumerate(mem_chunks):
                        chunks.append((qb, h, mem, ci == 0,
                                       ci == len(mem_chunks) - 1))

            pts = {}
            o_psum = {}
            pacc = {}
            oT_sb = {}
            ready = []
            ready_at = {}
            fifo = []

            def emit_front(i, ch):
                qb, h, mem, first, last = ch
                sd = s_ps.tile([128, 2 * QB], F32, tag="s", name=f"s_{i}")
                for nkb, qoff, soff in mem:
                    w = QB - qoff
                    nc.tensor.matmul(
                        sd[:, soff:soff + w],
                        lhsT=kT[h][:, nkb * KB:(nkb + 1) * KB],
                        rhs=qT[h][:, qb * QB + qoff:(qb + 1) * QB],
                        start=True,
                        stop=True,
                    )
                exp_end = mem[-1][2] + (QB - mem[-1][1])
                if causal and mem[0][0] >= 4 * qb:
                    # paired diagonal blocks: one combined-mask add
                    # (PSUM access -> must be DVE, gpsimd cannot touch PSUM)
                    mt = m1 if mem[0][1] == 0 else m2
                    nc.vector.tensor_add(
                        sd[:, :exp_end], sd[:, :exp_end], mt[:, :exp_end])
                elif not causal:
                    for nkb, qoff, soff in mem:
                        mt = mtpool.tile([128, QB], F32, tag="mt",
                                         name=f"mt_{i}_{soff}")
                        nc.sync.dma_start(
                            out=mt[:],
                            in_=maskT[nkb * KB:(nkb + 1) * KB,
                                      qb * QB:(qb + 1) * QB],
                        )
                        nc.vector.scalar_tensor_tensor(
                            out=sd[:, soff:soff + QB],
                            in0=mt[:],
                            scalar=1.0 / SCALE,
                            in1=sd[:, soff:soff + QB],
                            op0=mybir.AluOpType.mult,
                            op1=mybir.AluOpType.add,
                        )
                pt = ptpool.tile([128, 2 * QB], F16, tag="pt", name=f"pt_{i}")
                # one exp per chunk; the gap between diag members holds
                # garbage psum that is never consumed downstream
                nc.scalar.activation(
                    out=pt[:, :exp_end], in_=sd[:, :exp_end],
                    func=mybir.ActivationFunctionType.Exp,
                    scale=SCALE,
                )
                return pt

            def emit_back(ch, pt):
                qb, h, mem, first, last = ch
                if first:
                    o_psum[qb, h] = o_ps.tile([128, QB], F32, tag="o",
                                              name=f"o_{qb}_{h}")
                    pacc[qb, h] = paccp.tile([128, QB], F16, tag="pacc",
                                             name=f"pa_{qb}_{h}")
                oacc = o_psum[qb, h]
                pa = pacc[qb, h]
                for mi, (nkb, qoff, soff) in enumerate(mem):
                    w = QB - qoff
                    nc.tensor.matmul(
                        oacc[:, qoff:],
                        lhsT=v_sb[h][:, nkb * KB:(nkb + 1) * KB],
                        rhs=pt[:, soff:soff + w],
                        start=(first and mi == 0),
                        stop=(last and mi == len(mem) - 1),
                    )
                # exp-tile accumulation for the rowsum, split across engines:
                # gpsimd (slow but idle) folds member 1 into member 0's
                # columns inside the pt tile (SBUF-only), then ONE fast DVE
                # add merges the chunk into pacc.
                (n0, q0, _), (n1, q1, _) = mem
                w1 = QB - q1
                nc.gpsimd.tensor_add(
                    pt[:, q1 - q0:q1 - q0 + w1],
                    pt[:, q1 - q0:q1 - q0 + w1],
                    pt[:, QB:QB + w1],
                )
                w0 = QB - q0
                if first:
                    nc.vector.tensor_copy(out=pa[:], in_=pt[:, :QB])
                else:
                    nc.vector.tensor_add(
                        pa[:, q0:], pa[:, q0:], pt[:, :w0])
                if last:
                    finals.append((qb, h))

            def emit_final(qb, h):
                rps = y_ps.tile([128, QB], F32, tag="y", name=f"r_{qb}_{h}")
                nc.tensor.matmul(
                    rps[:], lhsT=ones[:], rhs=pacc.pop((qb, h))[:],
                    start=True, stop=True,
                )
                rinv = small.tile([128, QB], F32, tag="rinv",
                                  name=f"ri_{qb}_{h}")
                nc.vector.reciprocal_approx_fast(out=rinv[:], in_=rps[:])
                ot = otpool.tile([128, QB], F16, tag="ot",
                                 name=f"ot_{qb}_{h}")
                nc.vector.tensor_mul(ot[:], o_psum.pop((qb, h))[:], rinv[:])
                oT_sb[qb, h] = ot
                if h == HPC - 1:
                    ready.append(qb)
                    return True
                return False

            ysb_live = {}

            def emit_piece(qb, qs, dc):
                # one outproj piece: [128 rows x 512 cols of D] -- emitted one
                # per pipeline slot so the PE load is spread evenly and the
                # ACT/DVE evacuations never burst
                if dc == 0:
                    ysb_live[qb, qs] = ypool.tile([128, D], F16, tag="ysb",
                                                  name=f"ys_{qb}_{qs}")
                ysb = ysb_live[qb, qs]
                yps = y_ps.tile([128, QB], F32, tag="y",
                                name=f"y_{qb}_{qs}_{dc}")
                for h in range(HPC):
                    nc.tensor.matmul(
                        yps[:],
                        lhsT=oT_sb[qb, h][:, qs * 128:(qs + 1) * 128],
                        rhs=wo_sb[:, h * D + dc * QB: h * D + (dc + 1) * QB],
                        start=(h == 0),
                        stop=(h == HPC - 1),
                    )
                # psum -> fp16 evacuation, mostly DVE; ACT takes a quarter
                if dc == 3:
                    nc.scalar.copy(
                        out=ysb[:, dc * QB:(dc + 1) * QB], in_=yps[:])
                else:
                    nc.vector.tensor_copy(
                        out=ysb[:, dc * QB:(dc + 1) * QB], in_=yps[:])
                if dc == D // QB - 1:
                    del ysb_live[qb, qs]
                    row0 = qb * QB + qs * 128
                    nc.sync.dma_start(out=y[row0:row0 + 128, :], in_=ysb[:])

            SKEW = 3        # chunks of scores/exp lookahead ahead of PV
            DELAY = 5       # chunks between normalize and outproj start
            finals = []     # (qb, h) pairs whose pacc chain is fully
                            # emitted; their rowsum matmul is emitted one
                            # chunk later so the PE never waits on the
                            # gpsimd+DVE combine latency
            pending = []    # (qb, h, pushed_at)
            for i, ch in enumerate(chunks):
                pts[i] = emit_front(i, ch)
                if i >= SKEW:
                    emit_back(chunks[i - SKEW], pts.pop(i - SKEW))
                    while finals:
                        pending.append(finals.pop(0) + (i,))
                while pending and pending[0][2] < i:
                    qb0, h0, _ = pending.pop(0)
                    if emit_final(qb0, h0):
                        ready_at[ready[-1]] = i
                if fifo:
                    emit_piece(*fifo.pop(0))
                while ready and i - ready_at[ready[0]] >= DELAY:
                    qb0 = ready.pop(0)
                    fifo.extend((qb0, qs, dc) for qs in range(QB // 128)
                                for dc in range(D // QB))
            for j, ch in enumerate(chunks[-SKEW:]):
                emit_back(ch, pts.pop(len(chunks) - SKEW + j))
                if fifo:
                    emit_piece(*fifo.pop(0))
            for qb0, h0, _ in pending:
                emit_final(qb0, h0)
            while finals:
                emit_final(*finals.pop(0))
            # keep the PE (and its HAM clock gate) busy while the final
            # head's rowsum-reciprocal chain resolves
            warm2 = y_ps.tile([128, QB], F32, tag="y", name="warm2")
            for wi in range(6):
                nc.tensor.matmul(
                    warm2[:], lhsT=ones[:], rhs=warm_sb[:],
                    start=(wi == 0), stop=(wi == 5),
                )
            for qb0 in ready:
                fifo.extend((qb0, qs, dc) for qs in range(QB // 128)
                            for dc in range(D // QB))
            while fifo:
                emit_piece(*fifo.pop(0))

    nc.compile()
    return nc


_NC_CACHE: dict = {}


def _get_nc(causal: bool) -> bass.Bass:
    if causal not in _NC_CACHE:
        _NC_CACHE[causal] = build_nc(causal)
    return _NC_CACHE[causal]


def _make_in_maps(x, attn_mask, Wq, bq, Wk, bk, Wv, bv, Wo, bo, causal):
    xT = np.ascontiguousarray(x.T).astype(np.float16)
    maskT = None if causal else np.ascontiguousarray(attn_mask.T)
    in_maps = []
    for c in range(NCORES):
        sl = slice(c * CD, (c + 1) * CD)
        m = {
            "xT": xT,
            "wqT": np.ascontiguousarray(Wq[sl, :].T).astype(np.float16),
            "wkT": np.ascontiguousarray(Wk[sl, :].T).astype(np.float16),
            "wvT": np.ascontiguousarray(Wv[sl, :].T).astype(np.float16),
            "woT": np.ascontiguousarray(Wo[:, sl].T).astype(np.float16),
            "bq": np.ascontiguousarray(bq[sl]),
            "bk": np.ascontiguousarray(bk[sl]),
        }
        if maskT is not None:
            m["maskT"] = maskT
        in_maps.append(m)
    return in_maps


def _is_causal(attn_mask) -> bool:
    if attn_mask.shape != (N, N):
        return False
    expected = np.where(
        np.tril(np.ones((N, N), dtype=bool)), np.float32(0.0), np.float32(NEG)
    )
    return bool(np.array_equal(attn_mask, expected))


def run_spmd(in_maps, causal, **kwargs):
    nc = _get_nc(causal)
    return run_bass_kernel_spmd(nc, in_maps, core_ids=list(range(NCORES)), **kwargs)


def kernel(x, attn_mask, Wq, bq, Wk, bk, Wv, bv, Wo, bo):
    causal = _is_causal(np.asarray(attn_mask))
    in_maps = _make_in_maps(
        np.asarray(x, np.float32), np.asarray(attn_mask, np.float32),
        np.asarray(Wq, np.float32), np.asarray(bq, np.float32),
        np.asarray(Wk, np.float32), np.asarray(bk, np.float32),
        np.asarray(Wv, np.float32), np.asarray(bv, np.float32),
        np.asarray(Wo, np.float32), np.asarray(bo, np.float32),
        causal,
    )
    res = run_spmd(in_maps, causal)
    out = np.zeros((N, D), np.float32)
    for r in res.results:
        out += r["y"]
    # v bias folds into the output bias exactly (attn rows sum to 1); both
    # are added once here instead of per-core on device
    out += (np.asarray(bo, np.float32) + np.asarray(Wo, np.float32)
            @ np.asarray(bv, np.float32))[None, :]
    return out
